# revision 19
# baseline (speedup 1.0000x reference)
"""Trainium2 Bass kernel for an SNN layer (fc GEMM + leaky integrate-and-fire
scan + spike-count softmax), data-parallel over batch across 8 NeuronCores.

Computes, for x[64,128,4096], W[512,4096], b[512]:
    cur = einsum("bti,oi->bto", x, W) + b
    scan over t: mem' = 0.9*mem + cur_t - (mem > 1); spk_t = (mem' > 1)
    y = sum_t spk_t ; out = softmax(y, axis=-1)   -> [64, 512]

Strategy per core (batch shard of 8):
  - GEMM on PE in float32r (fp32 data, replicated PE mode: 1 cy/row when
    the moving free dim is >= 256 -- 4x the plain-fp32 rate, near-fp32
    numerics), W-stationary, producing cur^T tiles [no_part, rows] with
    rows t-major/b-minor so the scan reads contiguous 32-lane slices.
  - Time chunks [64, 32, 32]: the big first chunk gives the PE enough
    work to cover the W-stream phase of the DMA; PSUM = 4 banks for
    chunk 0 + 4 shared banks for chunks 1/2 (and the final transposes).
  - Bias added at PSUM eviction on ACT (Identity*1+bias, exact fp32).
  - LIF scan: one fused custom-DVE op per timestep on [128 x 32]:
        mem' = (mem * 0.9 + cur) - (mem > 1)
  - Spike count per chunk: is_gt into S (dead after scan) + t-reduce.
  - Softmax without max-subtraction (y <= 128, exp(y-64) is fp32-safe):
    PE transpose -> ACT Exp(bias=-64) with fused accum sum -> DVE
    reciprocal + scale -> DMA out.
"""

import numpy as np

B, T, NI, NO = 64, 128, 4096, 512
NCORES = 8
BS = B // NCORES            # 8 batch rows per core
R = BS * T                  # 1024 GEMM rows per core; row = t*BS + b
KT = NI // 128              # 32 contraction tiles
CN = NO // 128              # 4 output chunks of 128
TSPLIT = [64, 32, 32]       # timesteps per chunk (rows >= 256 keeps fp32r
NH = len(TSPLIT)            # at full PE rate)
TOFF = [sum(TSPLIT[:i]) for i in range(NH + 1)]
LAN = CN * BS               # 32 scan lanes per partition
BETA, THR = 0.9, 1.0
# k-tile DMA grouping: small leading groups so the first matmuls are
# gated on tiny transfers, bigger groups later; few issues total.
WGRP = [1, 3, 4, 8, 8, 8]
XGRP = [[1, 3, 4, 8, 8, 8], [8, 8, 16], [8, 8, 16]]
assert sum(WGRP) == KT and all(sum(g) == KT for g in XGRP)


def _k2g(grp):
    m = {}
    k = 0
    for g, n in enumerate(grp):
        for j in range(n):
            m[k] = (g, j)
            k += 1
    return m


WK2G = _k2g(WGRP)
XK2G = [_k2g(g) for g in XGRP]
# flat x layout: per (chunk h, group g) a [128, n_g*RH_h] block, row-major
XSIZES = [128 * n * (TSPLIT[h] * BS) for h in range(NH) for n in XGRP[h]]
XOFFS = [sum(XSIZES[:i]) for i in range(len(XSIZES) + 1)]
XTOT = XOFFS[-1]
XBLK = [sum(len(XGRP[i]) for i in range(h)) for h in range(NH)]

_PROG = None


def _lif_op():
    """Register (idempotently) the fused LIF-step custom DVE op:
    out = (Src0 * C0 + Src1) - (Src0 > C1)."""
    from concourse import dve_ops
    from concourse.dve_ops import DveOp
    from concourse.dve_spec import Spec, Src0, Src1, C0, C1, lower, _has_src1
    from concourse.dve_uop import DveOpSpec

    name = "LIF_STEP_ANT"
    for op in dve_ops.OPS:
        if op.name == name:
            return op

    spec = Spec(
        body=(Src0 * C0 + Src1) - (Src0 > C1),
        reference=lambda in0, in1, s0, s1, imm2: (
            (in0.astype(np.float32) * np.float32(s0) + in1)
            - (in0 > s1).astype(np.float32)
        ),
    )
    row = dve_ops._CUSTOM_DVE_ROW_BASE + len(dve_ops.OPS)
    assert row < 0x20, "custom DVE opcode rows exhausted"
    dve_ops._SUB_OPCODE_FOR_NAME[name] = row
    shas = {}
    for ver in ("v3", "v4"):
        uops = lower(spec, ver=ver)
        shas[ver] = DveOpSpec(
            name=name, opcode=row, uops=uops, rd1_en=_has_src1(spec)
        ).sha(ver)
    op = DveOp(name, spec, subdim=False, uops_sha=shas)
    dve_ops.OPS.append(op)
    dve_ops.CUSTOM_DVE_SPECS[name] = spec
    return op


def build_program(mm_dtype_name="float32r"):
    import concourse.bacc as bacc
    import concourse.mybir as mybir
    from concourse import tile
    from concourse.masks import make_identity

    f32 = mybir.dt.float32
    mmdt = getattr(mybir.dt, mm_dtype_name)
    lif = _lif_op()

    nc = bacc.Bacc("TRN2", target_bir_lowering=False, debug=False)

    xt_d = nc.dram_tensor("xt", [XTOT], mmdt, kind="ExternalInput").ap()
    wt_d = nc.dram_tensor("wt", [128, KT * NO], mmdt, kind="ExternalInput").ap()
    bt_d = nc.dram_tensor("bt", [128, CN], f32, kind="ExternalInput").ap()
    out_d = nc.dram_tensor("out", [BS, NO], f32, kind="ExternalOutput").ap()

    def x_dram(h, g):
        i = XBLK[h] + g
        return xt_d[XOFFS[i]:XOFFS[i + 1]].rearrange("(p q) -> p q", p=128)

    with tile.TileContext(nc) as tc:
        with (
            tc.tile_pool(name="wp", bufs=1) as wp,
            tc.tile_pool(name="xp", bufs=6) as xp,
            tc.tile_pool(name="sp", bufs=1) as sp,
            tc.tile_pool(name="cp", bufs=1) as cp,
            tc.tile_pool(name="accp", bufs=1, space="PSUM") as accp,
        ):
            # --- W / x chunk-0 DMAs, interleaved in need order ----------
            wg = [wp.tile([128, n * NO], mmdt, name=f"wg{g}", tag=f"wg{g}")
                  for g, n in enumerate(WGRP)]
            xg_tiles = {}

            def issue_x(h, g):
                n = XGRP[h][g]
                t_ = xp.tile([128, n * TSPLIT[h] * BS], mmdt,
                             name=f"xg{h}_{g}", tag="xg",
                             padded_shape=[128, 4096])
                nc.sync.dma_start(out=t_[:], in_=x_dram(h, g))
                xg_tiles[(h, g)] = t_

            WOFF = [sum(WGRP[:i]) for i in range(len(WGRP))]

            def issue_w(g):
                nc.sync.dma_start(
                    out=wg[g][:],
                    in_=wt_d[:, WOFF[g] * NO:(WOFF[g] + WGRP[g]) * NO])

            b_sb = cp.tile([128, CN], f32, name="bsb", tag="bsb")
            for g in range(len(WGRP)):
                issue_w(g)
                issue_x(0, g)
            # bias (tiny, first needed at chunk-0 eviction) after the bulk
            # stream so it does not delay the k-frontier
            nc.sync.dma_start(out=b_sb[:], in_=bt_d[:, :])
            zer = cp.tile([128, LAN], f32, name="zer", tag="zer")
            nc.gpsimd.memset(zer[:], 0.0)
            ident = cp.tile([128, 128], f32, name="ident", tag="ident")
            make_identity(nc, ident[:])
            # zero bf16 tile for PE keep-hot dummy matmuls (chunk 0 is
            # DMA-paced; idle gaps drop the PE to a low p-state clock)
            dident = cp.tile([128, 256], mybir.dt.bfloat16,
                             name="dident", tag="dident")
            nc.gpsimd.memset(dident[:], 0.0)

            S = [sp.tile([128, TSPLIT[h] * LAN], f32, name=f"s{h}", tag=f"s{h}")
                 for h in range(NH)]
            M = [sp.tile([128, TSPLIT[h] * LAN], f32, name=f"m{h}", tag=f"m{h}")
                 for h in range(NH)]
            yh = [cp.tile([128, LAN], f32, name=f"yh{h}", tag=f"yh{h}")
                  for h in range(NH)]

            # --- GEMM + scan + count, pipelined over time chunks --------
            # PSUM: chunk 0 owns 4 banks (a0_*); chunks 1/2 share 4 (a1_*).
            # Dummy ldweights keep the PE p-state clock high through chunk
            # 0's DMA-paced stretch (no output, so nothing to dead-code
            # away; every real matmul self-loads its own weights after).
            def dummy_mm():
                nc.tensor.ldweights(dident[:, :128])

            for _ in range(10):
                dummy_mm()
            for h in range(NH):
                ts = TSPLIT[h]
                rh = ts * BS
                pt = "a0" if h == 0 else "a1"
                psums = [accp.tile([128, rh], f32, name=f"acc{h}_{c}",
                                   tag=f"{pt}_{c}", padded_shape=[128, 512])
                         for c in range(CN)]
                # prefetch next chunk's x groups
                if h + 1 < NH:
                    for g in range(len(XGRP[h + 1])):
                        issue_x(h + 1, g)
                for k in range(KT):
                    xgi, xj = XK2G[h][k]
                    wgi, wj = WK2G[k]
                    xt_t = xg_tiles[(h, xgi)]
                    wgt = wg[wgi]
                    for c in range(CN):
                        nc.tensor.matmul(
                            psums[c][:],
                            lhsT=wgt[:, wj * NO + c * 128:
                                     wj * NO + (c + 1) * 128],
                            rhs=xt_t[:, xj * rh:(xj + 1) * rh],
                            start=(k == 0), stop=(k == KT - 1),
                        )
                    if h == 0 and k < KT - 1:
                        for _ in range(3):
                            dummy_mm()
                # evict PSUM + bias -> S[h] on ACT (Identity(x*1+bias) is
                # bit-exact fp32; keeps DVE free for the scan):
                # S[h][p, tt*LAN + c*BS + b] = psum_c[p, tt*BS+b] + bias_c[p]
                s_v = S[h].rearrange("p (t l) -> p t l", l=LAN)
                for c in range(CN):
                    nc.scalar.activation(
                        s_v[:, :, c * BS:(c + 1) * BS],
                        psums[c][:].rearrange("p (t b) -> p t b", b=BS),
                        mybir.ActivationFunctionType.Identity,
                        bias=b_sb[:, c:c + 1], scale=1.0,
                    )
                # LIF scan for this chunk's timesteps
                for tt in range(ts):
                    t = TOFF[h] + tt
                    cur = S[h][:, tt * LAN:(tt + 1) * LAN]
                    dst = M[h][:, tt * LAN:(tt + 1) * LAN]
                    if t == 0:
                        prev = zer[:]
                    elif tt == 0:
                        prev = M[h - 1][:, (TSPLIT[h - 1] - 1) * LAN:
                                        TSPLIT[h - 1] * LAN]
                    else:
                        prev = M[h][:, (tt - 1) * LAN: tt * LAN]
                    nc.vector._custom_dve(lif, out=dst, in0=prev, in1=cur,
                                          s0=BETA, s1=THR)
                # spike count for this chunk (S[h] is dead after the scan;
                # reuse it as the 0/1 spike buffer)
                nc.vector.tensor_scalar(out=S[h][:], in0=M[h][:], scalar1=THR,
                                        scalar2=None, op0=mybir.AluOpType.is_gt)
                nc.vector.tensor_reduce(
                    out=yh[h][:],
                    in_=S[h].rearrange("p (t l) -> p l t", l=LAN),
                    axis=mybir.AxisListType.X, op=mybir.AluOpType.add,
                )

            ysum = cp.tile([128, LAN], f32, name="ysum", tag="ysum")
            nc.vector.tensor_tensor(out=ysum[:], in0=yh[0][:], in1=yh[1][:],
                                    op=mybir.AluOpType.add)
            for h in range(2, NH):
                nc.vector.tensor_tensor(out=ysum[:], in0=ysum[:], in1=yh[h][:],
                                        op=mybir.AluOpType.add)

            # --- transpose y^T [no, b] -> [b, no], exp, normalize -------
            # y <= T = 128, so exp(y - 64) stays well inside fp32 range and
            # softmax needs no max-subtraction (exact same ratios).
            yb = cp.tile([BS, NO], f32, name="yb", tag="yb")
            sm4 = cp.tile([BS, CN], f32, name="sm4", tag="sm4")
            nb = cp.tile([BS, 1], f32, name="nb", tag="nb")
            nc.gpsimd.memset(nb[:], -64.0)
            y_v = ysum.rearrange("p (c b) -> p c b", b=BS)
            for c in range(CN):
                # PSUM banks a1_* are free once chunk 2 is evicted
                tp = accp.tile([BS, 128], f32, name=f"tp{c}",
                               tag=f"a1_{c}", padded_shape=[128, 512])
                nc.tensor.transpose(tp[:], y_v[:, c, :], ident[:])
                nc.scalar.activation(
                    yb[:, c * 128:(c + 1) * 128], tp[:],
                    mybir.ActivationFunctionType.Exp,
                    bias=nb[:], scale=1.0,
                    accum_out=sm4[:, c:c + 1],
                )
            sm = cp.tile([BS, 1], f32, name="sm", tag="sm")
            nc.vector.tensor_reduce(out=sm[:], in_=sm4[:],
                                    axis=mybir.AxisListType.X,
                                    op=mybir.AluOpType.add)
            rc = cp.tile([BS, 1], f32, name="rc", tag="rc")
            nc.vector.reciprocal(rc[:], sm[:])
            res = cp.tile([BS, NO], f32, name="res", tag="res")
            nc.vector.tensor_scalar(out=res[:], in0=yb[:], scalar1=rc[:],
                                    scalar2=None, op0=mybir.AluOpType.mult)

            nc.sync.dma_start(out=out_d[:], in_=res[:])

    nc.compile()
    return nc


def prep_inputs(x, W, b):
    """Host-side layout prep. Returns per-core in_maps."""
    x = np.asarray(x, dtype=np.float32)
    W = np.asarray(W, dtype=np.float32)
    b = np.asarray(b, dtype=np.float32)

    # wbig[p, k*NO + j] = W[j, k*128 + p]
    wbig = np.ascontiguousarray(
        W.T.reshape(KT, 128, NO).transpose(1, 0, 2)
    ).reshape(128, KT * NO)
    bt = np.ascontiguousarray(b.reshape(CN, 128).T)  # [128, CN]

    # x flat layout: blocks (h, g) of [128, XG, RH_h], row r = tt*BS + b_local
    xT = np.ascontiguousarray(x.transpose(2, 1, 0))  # [NI, T, B]
    in_maps = []
    for ci in range(NCORES):
        xs = xT[:, :, ci * BS:(ci + 1) * BS]                 # [NI, T, BS]
        xk = xs.reshape(KT, 128, T * BS)                     # [k, p, t*BS+b]
        flat = np.empty(XTOT, np.float32)
        i = 0
        for h in range(NH):
            r0, r1 = TOFF[h] * BS, TOFF[h + 1] * BS
            k0 = 0
            for n_g in XGRP[h]:
                blk = xk[k0:k0 + n_g, :, r0:r1]              # [n_g, 128, RH_h]
                flat[XOFFS[i]:XOFFS[i] + blk.size] = (
                    blk.transpose(1, 0, 2).reshape(-1))      # [p, k, r]
                k0 += n_g
                i += 1
        in_maps.append({"xt": flat, "wt": wbig, "bt": bt})
    return in_maps


def get_program():
    global _PROG
    if _PROG is None:
        _PROG = build_program()
    return _PROG


def kernel(x, W, b):
    from concourse import bass_utils

    nc = get_program()
    in_maps = prep_inputs(x, W, b)
    res = bass_utils.run_bass_kernel_spmd(nc, in_maps,
                                          core_ids=list(range(NCORES)))
    return np.concatenate([res.results[i]["out"] for i in range(NCORES)],
                          axis=0)


# revision 25
# speedup vs baseline: 1.1333x; 1.1333x over previous
"""Trainium2 Bass kernel for an SNN layer (fc GEMM + leaky integrate-and-fire
scan + spike-count softmax), data-parallel over batch across 8 NeuronCores.

Computes, for x[64,128,4096], W[512,4096], b[512]:
    cur = einsum("bti,oi->bto", x, W) + b
    scan over t: mem' = 0.9*mem + cur_t - (mem > 1); spk_t = (mem' > 1)
    y = sum_t spk_t ; out = softmax(y, axis=-1)   -> [64, 512]

Strategy per core (batch shard of 8):
  - GEMM on PE in float32r (fp32 data, replicated PE mode: 1 cy/row when
    the moving free dim is >= 256 -- 4x the plain-fp32 rate, near-fp32
    numerics), W-stationary, producing cur^T tiles [no_part, rows] with
    rows t-major/b-minor so the scan reads contiguous 32-lane slices.
  - Time chunks [64, 32, 32]: the big first chunk gives the PE enough
    work to cover the W-stream phase of the DMA; PSUM = 4 banks for
    chunk 0 + 4 shared banks for chunks 1/2 (and the final transposes).
  - Bias added at PSUM eviction on ACT (Identity*1+bias, exact fp32).
  - LIF scan: one fused custom-DVE op per timestep on [128 x 32]:
        mem' = (mem * 0.9 + cur) - (mem > 1)
  - Spike count per chunk: is_gt into S (dead after scan) + t-reduce.
  - Softmax without max-subtraction (y <= 128, exp(y-64) is fp32-safe):
    PE transpose -> ACT Exp(bias=-64) with fused accum sum -> DVE
    reciprocal + scale -> DMA out.
"""

import numpy as np

B, T, NI, NO = 64, 128, 4096, 512
NCORES = 8
BS = B // NCORES            # 8 batch rows per core
R = BS * T                  # 1024 GEMM rows per core; row = t*BS + b
KT = NI // 128              # 32 contraction tiles
CN = NO // 128              # 4 output chunks of 128
TSPLIT = [64, 32, 32]       # timesteps per chunk (rows >= 256 keeps fp32r
NH = len(TSPLIT)            # at full PE rate)
TOFF = [sum(TSPLIT[:i]) for i in range(NH + 1)]
LAN = CN * BS               # 32 scan lanes per partition
BETA, THR = 0.9, 1.0
# k-tile DMA grouping: each dynamic DMA pays ~1us of descriptor
# generation, so too-small leading groups throttle the stream; 4-tile
# (1MB) leading groups balance first-matmul latency vs stream rate.
WGRP = [4, 4, 8, 8, 8]
XGRP = [[4, 4, 8, 8, 8], [8, 8, 16], [8, 8, 16]]
assert sum(WGRP) == KT and all(sum(g) == KT for g in XGRP)


def _k2g(grp):
    m = {}
    k = 0
    for g, n in enumerate(grp):
        for j in range(n):
            m[k] = (g, j)
            k += 1
    return m


WK2G = _k2g(WGRP)
XK2G = [_k2g(g) for g in XGRP]
# flat x layout: per (chunk h, group g) a [128, n_g*RH_h] block, row-major
XSIZES = [128 * n * (TSPLIT[h] * BS) for h in range(NH) for n in XGRP[h]]
XOFFS = [sum(XSIZES[:i]) for i in range(len(XSIZES) + 1)]
XTOT = XOFFS[-1]
XBLK = [sum(len(XGRP[i]) for i in range(h)) for h in range(NH)]

_PROG = None


def _lif_op():
    """Register (idempotently) the fused LIF-step custom DVE op:
    out = (Src0 * C0 + Src1) - (Src0 > C1)."""
    from concourse import dve_ops
    from concourse.dve_ops import DveOp
    from concourse.dve_spec import Spec, Src0, Src1, C0, C1, lower, _has_src1
    from concourse.dve_uop import DveOpSpec

    name = "LIF_STEP_ANT"
    for op in dve_ops.OPS:
        if op.name == name:
            return op

    spec = Spec(
        body=(Src0 * C0 + Src1) - (Src0 > C1),
        reference=lambda in0, in1, s0, s1, imm2: (
            (in0.astype(np.float32) * np.float32(s0) + in1)
            - (in0 > s1).astype(np.float32)
        ),
    )
    row = dve_ops._CUSTOM_DVE_ROW_BASE + len(dve_ops.OPS)
    assert row < 0x20, "custom DVE opcode rows exhausted"
    dve_ops._SUB_OPCODE_FOR_NAME[name] = row
    shas = {}
    for ver in ("v3", "v4"):
        uops = lower(spec, ver=ver)
        shas[ver] = DveOpSpec(
            name=name, opcode=row, uops=uops, rd1_en=_has_src1(spec)
        ).sha(ver)
    op = DveOp(name, spec, subdim=False, uops_sha=shas)
    dve_ops.OPS.append(op)
    dve_ops.CUSTOM_DVE_SPECS[name] = spec
    return op


def build_program(mm_dtype_name="float32r"):
    import concourse.bacc as bacc
    import concourse.mybir as mybir
    from concourse import tile
    from concourse.masks import make_identity

    f32 = mybir.dt.float32
    mmdt = getattr(mybir.dt, mm_dtype_name)
    lif = _lif_op()

    nc = bacc.Bacc("TRN2", target_bir_lowering=False, debug=False)

    xt_d = nc.dram_tensor("xt", [XTOT], mmdt, kind="ExternalInput").ap()
    wt_d = nc.dram_tensor("wt", [128, KT * NO], mmdt, kind="ExternalInput").ap()
    bt_d = nc.dram_tensor("bt", [128, CN], f32, kind="ExternalInput").ap()
    out_d = nc.dram_tensor("out", [BS, NO], f32, kind="ExternalOutput").ap()

    def x_dram(h, g):
        i = XBLK[h] + g
        return xt_d[XOFFS[i]:XOFFS[i + 1]].rearrange("(p q) -> p q", p=128)

    with tile.TileContext(nc) as tc:
        with (
            tc.tile_pool(name="wp", bufs=1) as wp,
            tc.tile_pool(name="xp", bufs=6) as xp,
            tc.tile_pool(name="sp", bufs=1) as sp,
            tc.tile_pool(name="cp", bufs=1) as cp,
            tc.tile_pool(name="accp", bufs=1, space="PSUM") as accp,
        ):
            # --- W / x chunk-0 DMAs, interleaved in need order ----------
            wg = [wp.tile([128, n * NO], mmdt, name=f"wg{g}", tag=f"wg{g}")
                  for g, n in enumerate(WGRP)]
            xg_tiles = {}

            def issue_x(h, g):
                n = XGRP[h][g]
                t_ = xp.tile([128, n * TSPLIT[h] * BS], mmdt,
                             name=f"xg{h}_{g}", tag="xg",
                             padded_shape=[128, 4096])
                nc.sync.dma_start(out=t_[:], in_=x_dram(h, g))
                xg_tiles[(h, g)] = t_

            WOFF = [sum(WGRP[:i]) for i in range(len(WGRP))]

            def issue_w(g):
                nc.sync.dma_start(
                    out=wg[g][:],
                    in_=wt_d[:, WOFF[g] * NO:(WOFF[g] + WGRP[g]) * NO])

            b_sb = cp.tile([128, CN], f32, name="bsb", tag="bsb")
            for g in range(len(WGRP)):
                issue_w(g)
                issue_x(0, g)
            # bias (tiny, first needed at chunk-0 eviction) after the bulk
            # stream so it does not delay the k-frontier
            nc.sync.dma_start(out=b_sb[:], in_=bt_d[:, :])
            zer = cp.tile([128, LAN], f32, name="zer", tag="zer")
            nc.gpsimd.memset(zer[:], 0.0)
            ident = cp.tile([128, 128], f32, name="ident", tag="ident")
            make_identity(nc, ident[:])

            # S[h] = list of sub-tiles (the last chunk is split in two so
            # its scan can start after only half the eviction)
            SSPLIT = [1 if h < NH - 1 else 2 for h in range(NH)]
            S = [[sp.tile([128, TSPLIT[h] * LAN // SSPLIT[h]], f32,
                          name=f"s{h}_{j}", tag=f"s{h}_{j}")
                  for j in range(SSPLIT[h])] for h in range(NH)]
            M = [sp.tile([128, TSPLIT[h] * LAN], f32, name=f"m{h}", tag=f"m{h}")
                 for h in range(NH)]
            Q2 = sp.tile([128, TSPLIT[NH - 1] * LAN], f32, name="q2", tag="q2")
            yh = [cp.tile([128, LAN], f32, name=f"yh{h}", tag=f"yh{h}")
                  for h in range(NH)]

            # --- GEMM + scan + count, pipelined over time chunks --------
            # PSUM: alternate bank sets (a0 for chunks 0/2, a1 for chunk 1)
            # so no chunk's first matmul waits on the previous eviction.
            for h in range(NH):
                ts = TSPLIT[h]
                rh = ts * BS
                pt = "a0" if h % 2 == 0 else "a1"
                psums = [accp.tile([128, rh], f32, name=f"acc{h}_{c}",
                                   tag=f"{pt}_{c}", padded_shape=[128, 512])
                         for c in range(CN)]
                # prefetch next chunk's x groups
                if h + 1 < NH:
                    for g in range(len(XGRP[h + 1])):
                        issue_x(h + 1, g)
                for k in range(KT):
                    xgi, xj = XK2G[h][k]
                    wgi, wj = WK2G[k]
                    xt_t = xg_tiles[(h, xgi)]
                    wgt = wg[wgi]
                    for c in range(CN):
                        nc.tensor.matmul(
                            psums[c][:],
                            lhsT=wgt[:, wj * NO + c * 128:
                                     wj * NO + (c + 1) * 128],
                            rhs=xt_t[:, xj * rh:(xj + 1) * rh],
                            start=(k == 0), stop=(k == KT - 1),
                        )

                # evict PSUM + bias -> S[h] on ACT (Identity(x*1+bias) is
                # bit-exact fp32; keeps DVE free for the scan), sub-tile by
                # sub-tile so the scan can chase the first half:
                # S[h][p, tt*LAN + c*BS + b] = psum_c[p, tt*BS+b] + bias_c[p]
                tsub = ts // SSPLIT[h]
                for j in range(SSPLIT[h]):
                    s_v = S[h][j].rearrange("p (t l) -> p t l", l=LAN)
                    for c in range(CN):
                        nc.scalar.activation(
                            s_v[:, :, c * BS:(c + 1) * BS],
                            psums[c][:, j * tsub * BS:(j + 1) * tsub * BS]
                            .rearrange("p (t b) -> p t b", b=BS),
                            mybir.ActivationFunctionType.Identity,
                            bias=b_sb[:, c:c + 1], scale=1.0,
                        )
                # LIF scan for this chunk's timesteps
                for tt in range(ts):
                    t = TOFF[h] + tt
                    st = S[h][tt // tsub]
                    to = tt % tsub
                    cur = st[:, to * LAN:(to + 1) * LAN]
                    dst = M[h][:, tt * LAN:(tt + 1) * LAN]
                    if t == 0:
                        prev = zer[:]
                    elif tt == 0:
                        prev = M[h - 1][:, (TSPLIT[h - 1] - 1) * LAN:
                                        TSPLIT[h - 1] * LAN]
                    else:
                        prev = M[h][:, (tt - 1) * LAN: tt * LAN]
                    nc.vector._custom_dve(lif, out=dst, in0=prev, in1=cur,
                                          s0=BETA, s1=THR)
                # spike count for this chunk (S[h] / Q2 are scan-dead scratch)
                qt = S[h][0] if SSPLIT[h] == 1 else Q2
                nc.vector.tensor_scalar(out=qt[:], in0=M[h][:], scalar1=THR,
                                        scalar2=None, op0=mybir.AluOpType.is_gt)
                nc.vector.tensor_reduce(
                    out=yh[h][:],
                    in_=qt.rearrange("p (t l) -> p l t", l=LAN),
                    axis=mybir.AxisListType.X, op=mybir.AluOpType.add,
                )

            ysum = cp.tile([128, LAN], f32, name="ysum", tag="ysum")
            nc.vector.tensor_tensor(out=ysum[:], in0=yh[0][:], in1=yh[1][:],
                                    op=mybir.AluOpType.add)
            for h in range(2, NH):
                nc.vector.tensor_tensor(out=ysum[:], in0=ysum[:], in1=yh[h][:],
                                        op=mybir.AluOpType.add)

            # --- transpose y^T [no, b] -> [b, no], exp, normalize -------
            # y <= T = 128, so exp(y - 64) stays well inside fp32 range and
            # softmax needs no max-subtraction (exact same ratios).
            yb = cp.tile([BS, NO], f32, name="yb", tag="yb")
            sm4 = cp.tile([BS, CN], f32, name="sm4", tag="sm4")
            nb = cp.tile([BS, 1], f32, name="nb", tag="nb")
            nc.gpsimd.memset(nb[:], -64.0)
            y_v = ysum.rearrange("p (c b) -> p c b", b=BS)
            for c in range(CN):
                # PSUM banks a1_* are free once chunk 2 is evicted
                tp = accp.tile([BS, 128], f32, name=f"tp{c}",
                               tag=f"a1_{c}", padded_shape=[128, 512])
                nc.tensor.transpose(tp[:], y_v[:, c, :], ident[:])
                nc.scalar.activation(
                    yb[:, c * 128:(c + 1) * 128], tp[:],
                    mybir.ActivationFunctionType.Exp,
                    bias=nb[:], scale=1.0,
                    accum_out=sm4[:, c:c + 1],
                )
            sm = cp.tile([BS, 1], f32, name="sm", tag="sm")
            nc.vector.tensor_reduce(out=sm[:], in_=sm4[:],
                                    axis=mybir.AxisListType.X,
                                    op=mybir.AluOpType.add)
            rc = cp.tile([BS, 1], f32, name="rc", tag="rc")
            nc.vector.reciprocal(rc[:], sm[:])
            res = cp.tile([BS, NO], f32, name="res", tag="res")
            nc.vector.tensor_scalar(out=res[:], in0=yb[:], scalar1=rc[:],
                                    scalar2=None, op0=mybir.AluOpType.mult)

            nc.sync.dma_start(out=out_d[:], in_=res[:])

    nc.compile()
    return nc


def prep_inputs(x, W, b):
    """Host-side layout prep. Returns per-core in_maps."""
    x = np.asarray(x, dtype=np.float32)
    W = np.asarray(W, dtype=np.float32)
    b = np.asarray(b, dtype=np.float32)

    # wbig[p, k*NO + j] = W[j, k*128 + p]
    wbig = np.ascontiguousarray(
        W.T.reshape(KT, 128, NO).transpose(1, 0, 2)
    ).reshape(128, KT * NO)
    bt = np.ascontiguousarray(b.reshape(CN, 128).T)  # [128, CN]

    # x flat layout: blocks (h, g) of [128, XG, RH_h], row r = tt*BS + b_local
    xT = np.ascontiguousarray(x.transpose(2, 1, 0))  # [NI, T, B]
    in_maps = []
    for ci in range(NCORES):
        xs = xT[:, :, ci * BS:(ci + 1) * BS]                 # [NI, T, BS]
        xk = xs.reshape(KT, 128, T * BS)                     # [k, p, t*BS+b]
        flat = np.empty(XTOT, np.float32)
        i = 0
        for h in range(NH):
            r0, r1 = TOFF[h] * BS, TOFF[h + 1] * BS
            k0 = 0
            for n_g in XGRP[h]:
                blk = xk[k0:k0 + n_g, :, r0:r1]              # [n_g, 128, RH_h]
                flat[XOFFS[i]:XOFFS[i] + blk.size] = (
                    blk.transpose(1, 0, 2).reshape(-1))      # [p, k, r]
                k0 += n_g
                i += 1
        in_maps.append({"xt": flat, "wt": wbig, "bt": bt})
    return in_maps


def get_program():
    global _PROG
    if _PROG is None:
        _PROG = build_program()
    return _PROG


def kernel(x, W, b):
    from concourse import bass_utils

    nc = get_program()
    in_maps = prep_inputs(x, W, b)
    res = bass_utils.run_bass_kernel_spmd(nc, in_maps,
                                          core_ids=list(range(NCORES)))
    return np.concatenate([res.results[i]["out"] for i in range(NCORES)],
                          axis=0)


# revision 29
# speedup vs baseline: 1.1783x; 1.0397x over previous
"""Trainium2 Bass kernel for an SNN layer (fc GEMM + leaky integrate-and-fire
scan + spike-count softmax), data-parallel over batch across 8 NeuronCores.

Computes, for x[64,128,4096], W[512,4096], b[512]:
    cur = einsum("bti,oi->bto", x, W) + b
    scan over t: mem' = 0.9*mem + cur_t - (mem > 1); spk_t = (mem' > 1)
    y = sum_t spk_t ; out = softmax(y, axis=-1)   -> [64, 512]

Strategy per core (batch shard of 8):
  - GEMM on PE in float32r (fp32 data, replicated PE mode: 1 cy/row when
    the moving free dim is >= 256 -- 4x the plain-fp32 rate, near-fp32
    numerics), W-stationary, producing cur^T tiles [no_part, rows] with
    rows t-major/b-minor so the scan reads contiguous 32-lane slices.
  - Time chunks [64, 32, 32]: the big first chunk gives the PE enough
    work to cover the W-stream phase of the DMA; PSUM = 4 banks for
    chunk 0 + 4 shared banks for chunks 1/2 (and the final transposes).
  - Bias added at PSUM eviction on ACT (Identity*1+bias, exact fp32).
  - LIF scan: one fused custom-DVE op per timestep on [128 x 32]:
        mem' = (mem * 0.9 + cur) - (mem > 1)
  - Spike count per chunk: is_gt into S (dead after scan) + t-reduce.
  - Softmax without max-subtraction (y <= 128, exp(y-64) is fp32-safe):
    PE transpose -> ACT Exp(bias=-64) with fused accum sum -> DVE
    reciprocal + scale -> DMA out.
"""

import numpy as np

B, T, NI, NO = 64, 128, 4096, 512
NCORES = 8
BS = B // NCORES            # 8 batch rows per core
R = BS * T                  # 1024 GEMM rows per core; row = t*BS + b
KT = NI // 128              # 32 contraction tiles
CN = NO // 128              # 4 output chunks of 128
TSPLIT = [64, 32, 32]       # timesteps per chunk (rows >= 256 keeps fp32r
NH = len(TSPLIT)            # at full PE rate)
TOFF = [sum(TSPLIT[:i]) for i in range(NH + 1)]
LAN = CN * BS               # 32 scan lanes per partition
BETA, THR = 0.9, 1.0
# k-tile DMA grouping: each dynamic DMA pays ~1us of descriptor
# generation, so too-small leading groups throttle the stream; 4-tile
# (1MB) leading groups balance first-matmul latency vs stream rate.
WGRP = [4, 4, 8, 8, 4, 2, 2]
XGRP = [[4, 4, 8, 8, 4, 2, 2], [8, 8, 8, 8], [8, 8, 8, 8]]
assert sum(WGRP) == KT and all(sum(g) == KT for g in XGRP)


def _k2g(grp):
    m = {}
    k = 0
    for g, n in enumerate(grp):
        for j in range(n):
            m[k] = (g, j)
            k += 1
    return m


WK2G = _k2g(WGRP)
XK2G = [_k2g(g) for g in XGRP]
# flat x layout: per (chunk h, group g) a [128, n_g*RH_h] block, row-major
XSIZES = [128 * n * (TSPLIT[h] * BS) for h in range(NH) for n in XGRP[h]]
XOFFS = [sum(XSIZES[:i]) for i in range(len(XSIZES) + 1)]
XTOT = XOFFS[-1]
XBLK = [sum(len(XGRP[i]) for i in range(h)) for h in range(NH)]

_PROG = None


def _lif_op():
    """Register (idempotently) the fused LIF-step custom DVE op:
    out = (Src0 * C0 + Src1) - (Src0 > C1)."""
    from concourse import dve_ops
    from concourse.dve_ops import DveOp
    from concourse.dve_spec import Spec, Src0, Src1, C0, C1, lower, _has_src1
    from concourse.dve_uop import DveOpSpec

    name = "LIF_STEP_ANT"
    for op in dve_ops.OPS:
        if op.name == name:
            return op

    spec = Spec(
        body=(Src0 * C0 + Src1) - (Src0 > C1),
        reference=lambda in0, in1, s0, s1, imm2: (
            (in0.astype(np.float32) * np.float32(s0) + in1)
            - (in0 > s1).astype(np.float32)
        ),
    )
    row = dve_ops._CUSTOM_DVE_ROW_BASE + len(dve_ops.OPS)
    assert row < 0x20, "custom DVE opcode rows exhausted"
    dve_ops._SUB_OPCODE_FOR_NAME[name] = row
    shas = {}
    for ver in ("v3", "v4"):
        uops = lower(spec, ver=ver)
        shas[ver] = DveOpSpec(
            name=name, opcode=row, uops=uops, rd1_en=_has_src1(spec)
        ).sha(ver)
    op = DveOp(name, spec, subdim=False, uops_sha=shas)
    dve_ops.OPS.append(op)
    dve_ops.CUSTOM_DVE_SPECS[name] = spec
    return op


def build_program(mm_dtype_name="float32r"):
    import concourse.bacc as bacc
    import concourse.mybir as mybir
    from concourse import tile
    from concourse.masks import make_identity

    f32 = mybir.dt.float32
    mmdt = getattr(mybir.dt, mm_dtype_name)
    lif = _lif_op()

    nc = bacc.Bacc("TRN2", target_bir_lowering=False, debug=False)

    xt_d = nc.dram_tensor("xt", [XTOT], mmdt, kind="ExternalInput").ap()
    wt_d = nc.dram_tensor("wt", [128, KT * NO], mmdt, kind="ExternalInput").ap()
    bt_d = nc.dram_tensor("bt", [128, CN], f32, kind="ExternalInput").ap()
    out_d = nc.dram_tensor("out", [BS, NO], f32, kind="ExternalOutput").ap()

    def x_dram(h, g):
        i = XBLK[h] + g
        return xt_d[XOFFS[i]:XOFFS[i + 1]].rearrange("(p q) -> p q", p=128)

    with tile.TileContext(nc) as tc:
        with (
            tc.tile_pool(name="wp", bufs=1) as wp,
            tc.tile_pool(name="xp", bufs=6) as xp,
            tc.tile_pool(name="sp", bufs=1) as sp,
            tc.tile_pool(name="cp", bufs=1) as cp,
            tc.tile_pool(name="accp", bufs=1, space="PSUM") as accp,
        ):
            # --- W / x chunk-0 DMAs, interleaved in need order ----------
            wg = [wp.tile([128, n * NO], mmdt, name=f"wg{g}", tag=f"wg{g}")
                  for g, n in enumerate(WGRP)]
            xg_tiles = {}

            def issue_x(h, g):
                n = XGRP[h][g]
                t_ = xp.tile([128, n * TSPLIT[h] * BS], mmdt,
                             name=f"xg{h}_{g}", tag="xg",
                             padded_shape=[128, 4096])
                nc.sync.dma_start(out=t_[:], in_=x_dram(h, g))
                xg_tiles[(h, g)] = t_

            WOFF = [sum(WGRP[:i]) for i in range(len(WGRP))]

            def issue_w(g):
                nc.sync.dma_start(
                    out=wg[g][:],
                    in_=wt_d[:, WOFF[g] * NO:(WOFF[g] + WGRP[g]) * NO])

            b_sb = cp.tile([128, CN], f32, name="bsb", tag="bsb")
            for g in range(len(WGRP)):
                issue_w(g)
                issue_x(0, g)
            # bias (tiny, first needed at chunk-0 eviction) after the bulk
            # stream so it does not delay the k-frontier
            nc.sync.dma_start(out=b_sb[:], in_=bt_d[:, :])
            zer = cp.tile([128, LAN], f32, name="zer", tag="zer")
            nc.gpsimd.memset(zer[:], 0.0)
            ident = cp.tile([128, 128], f32, name="ident", tag="ident")
            make_identity(nc, ident[:])

            # S[h] = list of sub-tiles (first/last chunks split in two so
            # their scans can start after only half the eviction)
            SSPLIT = [2, 1, 2]
            S = [[sp.tile([128, TSPLIT[h] * LAN // SSPLIT[h]], f32,
                          name=f"s{h}_{j}", tag=f"s{h}_{j}")
                  for j in range(SSPLIT[h])] for h in range(NH)]
            M = [sp.tile([128, TSPLIT[h] * LAN], f32, name=f"m{h}", tag=f"m{h}")
                 for h in range(NH)]
            QW = max(TSPLIT[h] * LAN for h in range(NH) if SSPLIT[h] > 1)
            Q2 = sp.tile([128, QW], f32, name="q2", tag="q2")
            yh = [cp.tile([128, LAN], f32, name=f"yh{h}", tag=f"yh{h}")
                  for h in range(NH)]

            # --- GEMM + scan + count, pipelined over time chunks --------
            # PSUM: alternate bank sets (a0 for chunks 0/2, a1 for chunk 1)
            # so no chunk's first matmul waits on the previous eviction.
            for h in range(NH):
                ts = TSPLIT[h]
                rh = ts * BS
                pt = "a0" if h % 2 == 0 else "a1"
                psums = [accp.tile([128, rh], f32, name=f"acc{h}_{c}",
                                   tag=f"{pt}_{c}", padded_shape=[128, 512])
                         for c in range(CN)]
                # prefetch next chunk's x groups
                if h + 1 < NH:
                    for g in range(len(XGRP[h + 1])):
                        issue_x(h + 1, g)
                for k in range(KT):
                    xgi, xj = XK2G[h][k]
                    wgi, wj = WK2G[k]
                    xt_t = xg_tiles[(h, xgi)]
                    wgt = wg[wgi]
                    for c in range(CN):
                        nc.tensor.matmul(
                            psums[c][:],
                            lhsT=wgt[:, wj * NO + c * 128:
                                     wj * NO + (c + 1) * 128],
                            rhs=xt_t[:, xj * rh:(xj + 1) * rh],
                            start=(k == 0), stop=(k == KT - 1),
                        )

                # evict PSUM + bias -> S[h] on ACT (Identity(x*1+bias) is
                # bit-exact fp32; keeps DVE free for the scan), sub-tile by
                # sub-tile so the scan can chase the first half:
                # S[h][p, tt*LAN + c*BS + b] = psum_c[p, tt*BS+b] + bias_c[p]
                tsub = ts // SSPLIT[h]
                for j in range(SSPLIT[h]):
                    s_v = S[h][j].rearrange("p (t l) -> p t l", l=LAN)
                    for c in range(CN):
                        nc.scalar.activation(
                            s_v[:, :, c * BS:(c + 1) * BS],
                            psums[c][:, j * tsub * BS:(j + 1) * tsub * BS]
                            .rearrange("p (t b) -> p t b", b=BS),
                            mybir.ActivationFunctionType.Identity,
                            bias=b_sb[:, c:c + 1], scale=1.0,
                        )
                # LIF scan for this chunk's timesteps
                for tt in range(ts):
                    t = TOFF[h] + tt
                    st = S[h][tt // tsub]
                    to = tt % tsub
                    cur = st[:, to * LAN:(to + 1) * LAN]
                    dst = M[h][:, tt * LAN:(tt + 1) * LAN]
                    if t == 0:
                        prev = zer[:]
                    elif tt == 0:
                        prev = M[h - 1][:, (TSPLIT[h - 1] - 1) * LAN:
                                        TSPLIT[h - 1] * LAN]
                    else:
                        prev = M[h][:, (tt - 1) * LAN: tt * LAN]
                    nc.vector._custom_dve(lif, out=dst, in0=prev, in1=cur,
                                          s0=BETA, s1=THR)
                # spike count for this chunk (S[h] / Q2 are scan-dead scratch)
                qt = (S[h][0] if SSPLIT[h] == 1 else Q2)[:, :ts * LAN]
                nc.vector.tensor_scalar(out=qt, in0=M[h][:], scalar1=THR,
                                        scalar2=None, op0=mybir.AluOpType.is_gt)
                nc.vector.tensor_reduce(
                    out=yh[h][:],
                    in_=qt.rearrange("p (t l) -> p l t", l=LAN),
                    axis=mybir.AxisListType.X, op=mybir.AluOpType.add,
                )

            ysum = cp.tile([128, LAN], f32, name="ysum", tag="ysum")
            nc.vector.tensor_tensor(out=ysum[:], in0=yh[0][:], in1=yh[1][:],
                                    op=mybir.AluOpType.add)
            for h in range(2, NH):
                nc.vector.tensor_tensor(out=ysum[:], in0=ysum[:], in1=yh[h][:],
                                        op=mybir.AluOpType.add)

            # --- transpose y^T [no, b] -> [b, no], exp, normalize -------
            # y <= T = 128, so exp(y - 64) stays well inside fp32 range and
            # softmax needs no max-subtraction (exact same ratios).
            yb = cp.tile([BS, NO], f32, name="yb", tag="yb")
            sm4 = cp.tile([BS, CN], f32, name="sm4", tag="sm4")
            nb = cp.tile([BS, 1], f32, name="nb", tag="nb")
            nc.gpsimd.memset(nb[:], -64.0)
            y_v = ysum.rearrange("p (c b) -> p c b", b=BS)
            for c in range(CN):
                # PSUM banks a1_* are free once chunk 2 is evicted
                tp = accp.tile([BS, 128], f32, name=f"tp{c}",
                               tag=f"a1_{c}", padded_shape=[128, 512])
                nc.tensor.transpose(tp[:], y_v[:, c, :], ident[:])
                nc.scalar.activation(
                    yb[:, c * 128:(c + 1) * 128], tp[:],
                    mybir.ActivationFunctionType.Exp,
                    bias=nb[:], scale=1.0,
                    accum_out=sm4[:, c:c + 1],
                )
            sm = cp.tile([BS, 1], f32, name="sm", tag="sm")
            nc.vector.tensor_reduce(out=sm[:], in_=sm4[:],
                                    axis=mybir.AxisListType.X,
                                    op=mybir.AluOpType.add)
            rc = cp.tile([BS, 1], f32, name="rc", tag="rc")
            nc.vector.reciprocal(rc[:], sm[:])
            res = cp.tile([BS, NO], f32, name="res", tag="res")
            nc.vector.tensor_scalar(out=res[:], in0=yb[:], scalar1=rc[:],
                                    scalar2=None, op0=mybir.AluOpType.mult)

            nc.sync.dma_start(out=out_d[:], in_=res[:])

    nc.compile()
    return nc


def prep_inputs(x, W, b):
    """Host-side layout prep. Returns per-core in_maps."""
    x = np.asarray(x, dtype=np.float32)
    W = np.asarray(W, dtype=np.float32)
    b = np.asarray(b, dtype=np.float32)

    # wbig[p, k*NO + j] = W[j, k*128 + p]
    wbig = np.ascontiguousarray(
        W.T.reshape(KT, 128, NO).transpose(1, 0, 2)
    ).reshape(128, KT * NO)
    bt = np.ascontiguousarray(b.reshape(CN, 128).T)  # [128, CN]

    # x flat layout: blocks (h, g) of [128, XG, RH_h], row r = tt*BS + b_local
    xT = np.ascontiguousarray(x.transpose(2, 1, 0))  # [NI, T, B]
    in_maps = []
    for ci in range(NCORES):
        xs = xT[:, :, ci * BS:(ci + 1) * BS]                 # [NI, T, BS]
        xk = xs.reshape(KT, 128, T * BS)                     # [k, p, t*BS+b]
        flat = np.empty(XTOT, np.float32)
        i = 0
        for h in range(NH):
            r0, r1 = TOFF[h] * BS, TOFF[h + 1] * BS
            k0 = 0
            for n_g in XGRP[h]:
                blk = xk[k0:k0 + n_g, :, r0:r1]              # [n_g, 128, RH_h]
                flat[XOFFS[i]:XOFFS[i] + blk.size] = (
                    blk.transpose(1, 0, 2).reshape(-1))      # [p, k, r]
                k0 += n_g
                i += 1
        in_maps.append({"xt": flat, "wt": wbig, "bt": bt})
    return in_maps


def get_program():
    global _PROG
    if _PROG is None:
        _PROG = build_program()
    return _PROG


def kernel(x, W, b):
    from concourse import bass_utils

    nc = get_program()
    in_maps = prep_inputs(x, W, b)
    res = bass_utils.run_bass_kernel_spmd(nc, in_maps,
                                          core_ids=list(range(NCORES)))
    return np.concatenate([res.results[i]["out"] for i in range(NCORES)],
                          axis=0)
